# revision 3
# baseline (speedup 1.0000x reference)
"""Trainium2 Bass kernel for windowed Conv1d(k=3) + sigmoid gating.

Reference computation (B=16, T=960, D=1024, W=10):
  windows of size 10 are conv'd independently with per-window zero pad 1:
    cnn[r, d] = sum_{k,c} conv_w[d, c, k] * xpad[r + k - 1, c] + conv_b[d]
    out = cnn * sigmoid(cnn @ gate_w.T + gate_b)

Strategy: pure data parallelism over the 8 NeuronCores (2 batches per
core, 192 windows = 1920 rows each). All layout work happens on the host:
  - x is window-padded and transposed to [D, 12, 192] (channel-major,
    tap-position-major) so the conv becomes 24 accumulating bf16 matmuls
    per output tile with the contraction dim on SBUF partitions; the 3
    taps are column offsets of k*192 elements (always 4B aligned).
  - conv_w / gate_w are pre-transposed into lhsT layout, bf16.
  - The kernel computes everything in transposed space [d, r] and writes a
    transposed f32 output that the host transposes back.
"""

import numpy as np
import ml_dtypes

import concourse.bacc as bacc
import concourse.bass as bass
import concourse.tile as tile
from concourse import mybir
from concourse.bass_utils import run_bass_kernel_spmd

BF16 = ml_dtypes.bfloat16

B, T, D, W = 16, 960, 1024, 10
NCORES = 8
BC = B // NCORES            # batches per core
NWIN = BC * T // W          # windows per core (192)
RC = NWIN * W               # output rows per core (1920)
PW = W + 2                  # padded window length (12)
NG = 4                      # column groups per core
GWIN = NWIN // NG           # windows per group (48)
GN = GWIN * W               # output columns per group (480)
NCH = D // 128              # 128-partition chunks of D (8)


def _build():
    nc = bacc.Bacc("TRN2", target_bir_lowering=False, debug=False)

    xpt = nc.dram_tensor("xpt", [D, PW * NWIN], mybir.dt.bfloat16, kind="ExternalInput")
    cw = nc.dram_tensor("cw", [3 * NCH, 128, D], mybir.dt.bfloat16, kind="ExternalInput")
    gw = nc.dram_tensor("gw", [NCH, 128, D], mybir.dt.bfloat16, kind="ExternalInput")
    cb = nc.dram_tensor("cb", [128, NCH], mybir.dt.float32, kind="ExternalInput")
    gb = nc.dram_tensor("gb", [128, NCH], mybir.dt.float32, kind="ExternalInput")
    outT = nc.dram_tensor("outT", [D, RC], mybir.dt.float32, kind="ExternalOutput")

    with tile.TileContext(nc) as tc:
        with (
            tc.tile_pool(name="consts", bufs=1) as consts,
            tc.tile_pool(name="work", bufs=3) as work,
            tc.tile_pool(name="cnn", bufs=2) as cnnp,
            tc.tile_pool(name="cpsum", bufs=2, space="PSUM") as cpsum,
            tc.tile_pool(name="gpsum", bufs=2, space="PSUM") as gpsum,
        ):
            cb_sb = consts.tile([128, NCH], mybir.dt.float32, tag="cb")
            nc.sync.dma_start(cb_sb[:], cb[:])
            gb_sb = consts.tile([128, NCH], mybir.dt.float32, tag="gb")
            nc.sync.dma_start(gb_sb[:], gb[:])

            xpt_sb = []
            for c in range(NCH):
                t = consts.tile([128, PW * NWIN], mybir.dt.bfloat16, tag=f"x{c}")
                nc.sync.dma_start(t[:], xpt[c * 128:(c + 1) * 128, :])
                xpt_sb.append(t)

            cw_sb = []
            for j in range(3 * NCH):
                t = consts.tile([128, D], mybir.dt.bfloat16, tag=f"cw{j}")
                nc.sync.dma_start(t[:], cw[j])
                cw_sb.append(t)

            gw_sb = []
            for j in range(NCH):
                t = consts.tile([128, D], mybir.dt.bfloat16, tag=f"gw{j}")
                nc.sync.dma_start(t[:], gw[j])
                gw_sb.append(t)

            for g in range(NG):
                # conv: cnnT[d, (t, win)] = sum_{c,k} cw[k][c, d] * xpt[c, t+k, win]
                cnnT = []
                for dck in range(NCH):
                    ps = cpsum.tile([128, GN], mybir.dt.float32, tag="cps")
                    n_mm = 0
                    for ck in range(NCH):
                        xg = xpt_sb[ck][:].rearrange("p (t w) -> p t w", w=NWIN)
                        for k in range(3):
                            nc.tensor.matmul(
                                ps[:],
                                cw_sb[k * NCH + ck][:, dck * 128:(dck + 1) * 128],
                                xg[:, k:k + W, g * GWIN:(g + 1) * GWIN],
                                start=(n_mm == 0),
                                stop=(n_mm == 3 * NCH - 1),
                            )
                            n_mm += 1
                    ct = cnnp.tile([128, GN], mybir.dt.bfloat16, tag=f"cnn{dck}")
                    nc.vector.tensor_scalar_add(ct[:], ps[:], cb_sb[:, dck:dck + 1])
                    cnnT.append(ct)

                # gate: gateT[e, r] = sigmoid(sum_d gw[d, e] * cnnT[d, r] + gb[e])
                for eck in range(NCH):
                    ps2 = gpsum.tile([128, GN], mybir.dt.float32, tag="gps")
                    for dck in range(NCH):
                        nc.tensor.matmul(
                            ps2[:],
                            gw_sb[dck][:, eck * 128:(eck + 1) * 128],
                            cnnT[dck][:],
                            start=(dck == 0),
                            stop=(dck == NCH - 1),
                        )
                    gt = work.tile([128, GN], mybir.dt.bfloat16, tag="gate")
                    nc.scalar.activation(
                        gt[:], ps2[:], mybir.ActivationFunctionType.Sigmoid,
                        bias=gb_sb[:, eck:eck + 1],
                    )
                    ot = work.tile([128, GN], mybir.dt.float32, tag="out")
                    nc.vector.tensor_mul(ot[:], cnnT[eck][:], gt[:])
                    nc.sync.dma_start(
                        outT[eck * 128:(eck + 1) * 128, g * GN:(g + 1) * GN], ot[:]
                    )
    nc.compile()
    return nc


def _prep_core_input(x_shard, cw_host, gw_host, cb_host, gb_host):
    # x_shard: [BC, T, D] -> padded transposed [D, PW, NWIN] -> [D, PW*NWIN]
    xs = x_shard.reshape(NWIN, W, D)
    xp = np.zeros((D, PW, NWIN), np.float32)
    xp[:, 1:1 + W, :] = xs.transpose(2, 1, 0)
    xpt_host = np.ascontiguousarray(xp.reshape(D, PW * NWIN)).astype(BF16)
    return {"xpt": xpt_host, "cw": cw_host, "gw": gw_host,
            "cb": cb_host, "gb": gb_host}


def _prep_in_maps(x, conv_w, conv_b, gate_w, gate_b):
    # lhsT layouts: cw[k*8+ck][c, d] = conv_w[d, c, k]; gw[dck][d, e] = gate_w[e, d]
    cw_host = np.ascontiguousarray(conv_w.transpose(2, 1, 0)).reshape(
        3 * NCH, 128, D).astype(BF16)
    gw_host = np.ascontiguousarray(gate_w.T).reshape(NCH, 128, D).astype(BF16)
    cb_host = np.ascontiguousarray(conv_b.reshape(NCH, 128).T).astype(np.float32)
    gb_host = np.ascontiguousarray(gate_b.reshape(NCH, 128).T).astype(np.float32)
    return [
        _prep_core_input(x[BC * i:BC * (i + 1)], cw_host, gw_host, cb_host, gb_host)
        for i in range(NCORES)
    ]


def _unshard_core(o):
    # o: [D, RC] with columns ordered (group, t, win) -> [BC, T, D]
    return (o.reshape(D, NG, W, GWIN).transpose(1, 3, 2, 0)
             .reshape(NWIN, W, D).reshape(BC, T, D))


_NC_CACHE = None


def kernel(x, conv_w, conv_b, gate_w, gate_b):
    global _NC_CACHE
    x = np.asarray(x, np.float32)
    conv_w = np.asarray(conv_w, np.float32)
    conv_b = np.asarray(conv_b, np.float32)
    gate_w = np.asarray(gate_w, np.float32)
    gate_b = np.asarray(gate_b, np.float32)

    in_maps = _prep_in_maps(x, conv_w, conv_b, gate_w, gate_b)
    if _NC_CACHE is None:
        _NC_CACHE = _build()
    res = run_bass_kernel_spmd(_NC_CACHE, in_maps, core_ids=list(range(NCORES))).results

    out = np.empty((B, T, D), np.float32)
    for i in range(NCORES):
        out[BC * i:BC * (i + 1)] = _unshard_core(np.asarray(res[i]["outT"]))
    return out


# revision 7
# speedup vs baseline: 1.0788x; 1.0788x over previous
"""Trainium2 Bass kernel for windowed Conv1d(k=3) + sigmoid gating.

Reference computation (B=16, T=960, D=1024, W=10):
  windows of size 10 are conv'd independently with per-window zero pad 1:
    cnn[r, d] = sum_{k,c} conv_w[d, c, k] * xpad[r + k - 1, c] + conv_b[d]
    out = cnn * sigmoid(cnn @ gate_w.T + gate_b)

Strategy: pure data parallelism over the 8 NeuronCores (2 batches per
core, 192 windows = 1920 rows each). All layout work happens on the host:
  - x is window-padded and transposed to [D, 12, 192] (channel-major,
    tap-position-major) so the conv becomes 24 accumulating bf16 matmuls
    per output tile with the contraction dim on SBUF partitions; the 3
    taps are column offsets of k*192 elements (always 4B aligned).
  - conv_w / gate_w are pre-transposed into lhsT layout, bf16.
  - The kernel computes everything in transposed space [d, r] and writes a
    transposed f32 output that the host transposes back.
"""

import numpy as np
import ml_dtypes

import concourse.bacc as bacc
import concourse.bass as bass
import concourse.tile as tile
from concourse import mybir
from concourse.bass_utils import run_bass_kernel_spmd

BF16 = ml_dtypes.bfloat16

B, T, D, W = 16, 960, 1024, 10
NCORES = 8
BC = B // NCORES            # batches per core
NWIN = BC * T // W          # windows per core (192)
RC = NWIN * W               # output rows per core (1920)
PW = W + 2                  # padded window length (12)
NG = 4                      # column groups per core
GWIN = NWIN // NG           # windows per group (48)
GN = GWIN * W               # output columns per group (480)
NCH = D // 128              # 128-partition chunks of D (8)


def _build():
    nc = bacc.Bacc("TRN2", target_bir_lowering=False, debug=False)

    # xpt: [c, group, t*win] window-padded transposed input
    xpt = nc.dram_tensor("xpt", [D, NG, PW * GWIN], mybir.dt.bfloat16,
                         kind="ExternalInput")
    # cwr[dck]: [c_in_chunk, (k*NCH+ck)*128 + d_in_chunk] conv lhsT blocks
    cwr = nc.dram_tensor("cwr", [NCH, 128, 3 * NCH * 128], mybir.dt.bfloat16,
                         kind="ExternalInput")
    # gwr[eck]: [d_in_chunk, dck*128 + e_in_chunk] gate lhsT blocks
    gwr = nc.dram_tensor("gwr", [NCH, 128, NCH * 128], mybir.dt.bfloat16,
                         kind="ExternalInput")
    cb = nc.dram_tensor("cb", [128, NCH], mybir.dt.float32, kind="ExternalInput")
    gb = nc.dram_tensor("gb", [128, NCH], mybir.dt.float32, kind="ExternalInput")
    outT = nc.dram_tensor("outT", [D, RC], mybir.dt.float32, kind="ExternalOutput")

    with tile.TileContext(nc) as tc:
        with (
            tc.tile_pool(name="consts", bufs=1) as consts,
            tc.tile_pool(name="work", bufs=3) as work,
            tc.tile_pool(name="cnn", bufs=2) as cnnp,
            tc.tile_pool(name="cpsum", bufs=2, space="PSUM") as cpsum,
            tc.tile_pool(name="gpsum", bufs=2, space="PSUM") as gpsum,
        ):
            # DMA issue order = first-use order: x group 0, conv weights for
            # d-chunk 0, biases, remaining conv weights, gate weights, then
            # the remaining x groups stream in under the compute.
            xpt_sb = [[None] * NG for _ in range(NCH)]

            def load_x(c, g):
                t = consts.tile([128, PW * GWIN], mybir.dt.bfloat16, tag=f"x{c}g{g}")
                nc.sync.dma_start(t[:], xpt[c * 128:(c + 1) * 128, g])
                xpt_sb[c][g] = t

            for c in range(NCH):
                load_x(c, 0)

            cwr_sb = [None] * NCH

            def load_cw(dck):
                t = consts.tile([128, 3 * NCH * 128], mybir.dt.bfloat16, tag=f"cw{dck}")
                nc.sync.dma_start(t[:], cwr[dck])
                cwr_sb[dck] = t

            load_cw(0)

            cb_sb = consts.tile([128, NCH], mybir.dt.float32, tag="cb")
            nc.sync.dma_start(cb_sb[:], cb[:])
            gb_sb = consts.tile([128, NCH], mybir.dt.float32, tag="gb")
            nc.sync.dma_start(gb_sb[:], gb[:])

            for dck in range(1, NCH):
                load_cw(dck)

            gwr_sb = []
            for eck in range(NCH):
                t = consts.tile([128, NCH * 128], mybir.dt.bfloat16, tag=f"gw{eck}")
                nc.sync.dma_start(t[:], gwr[eck])
                gwr_sb.append(t)

            for g in range(1, NG):
                for c in range(NCH):
                    load_x(c, g)

            for g in range(NG):
                # conv: cnnT[d, (t, win)] = sum_{c,k} cw[k][c, d] * xpt[c, t+k, win]
                cnnT = []
                for dck in range(NCH):
                    ps = cpsum.tile([128, GN], mybir.dt.float32, tag="cps")
                    n_mm = 0
                    for ck in range(NCH):
                        xg = xpt_sb[ck][g][:].rearrange("p (t w) -> p t w", w=GWIN)
                        for k in range(3):
                            j = k * NCH + ck
                            nc.tensor.matmul(
                                ps[:],
                                cwr_sb[dck][:, j * 128:(j + 1) * 128],
                                xg[:, k:k + W, :],
                                start=(n_mm == 0),
                                stop=(n_mm == 3 * NCH - 1),
                            )
                            n_mm += 1
                    ct = cnnp.tile([128, GN], mybir.dt.bfloat16, tag=f"cnn{dck}")
                    nc.vector.tensor_scalar_add(ct[:], ps[:], cb_sb[:, dck:dck + 1])
                    cnnT.append(ct)

                # gate: gateT[e, r] = sigmoid(sum_d gw[d, e] * cnnT[d, r] + gb[e])
                for eck in range(NCH):
                    ps2 = gpsum.tile([128, GN], mybir.dt.float32, tag="gps")
                    for dck in range(NCH):
                        nc.tensor.matmul(
                            ps2[:],
                            gwr_sb[eck][:, dck * 128:(dck + 1) * 128],
                            cnnT[dck][:],
                            start=(dck == 0),
                            stop=(dck == NCH - 1),
                        )
                    gt = work.tile([128, GN], mybir.dt.bfloat16, tag="gate")
                    nc.scalar.activation(
                        gt[:], ps2[:], mybir.ActivationFunctionType.Sigmoid,
                        bias=gb_sb[:, eck:eck + 1],
                    )
                    ot = work.tile([128, GN], mybir.dt.float32, tag="out")
                    nc.vector.tensor_mul(ot[:], cnnT[eck][:], gt[:])
                    nc.sync.dma_start(
                        outT[eck * 128:(eck + 1) * 128, g * GN:(g + 1) * GN], ot[:]
                    )
    nc.compile()
    return nc


def _prep_core_input(x_shard, cw_host, gw_host, cb_host, gb_host):
    # x_shard: [BC, T, D] -> padded transposed [D, NG, PW, GWIN]
    xs = x_shard.reshape(NG, GWIN, W, D)
    xp = np.zeros((D, NG, PW, GWIN), np.float32)
    xp[:, :, 1:1 + W, :] = xs.transpose(3, 0, 2, 1)
    xpt_host = np.ascontiguousarray(xp).astype(BF16).reshape(D, NG, PW * GWIN)
    return {"xpt": xpt_host, "cwr": cw_host, "gwr": gw_host,
            "cb": cb_host, "gb": gb_host}


def _prep_in_maps(x, conv_w, conv_b, gate_w, gate_b):
    # conv lhsT blocks: cwr[dck][cc, (k*NCH+ck)*128 + dd] = conv_w[dck*128+dd, ck*128+cc, k]
    cwt = conv_w.transpose(2, 1, 0).reshape(3, NCH, 128, NCH, 128)  # [k, ck, cc, dck, dd]
    cw_host = np.ascontiguousarray(cwt.transpose(3, 2, 0, 1, 4)).reshape(
        NCH, 128, 3 * NCH * 128).astype(BF16)
    # gate lhsT blocks: gwr[eck][dd, dck*128 + ee] = gate_w[eck*128+ee, dck*128+dd]
    gwt = gate_w.T.reshape(NCH, 128, NCH, 128)  # [dck, dd, eck, ee]
    gw_host = np.ascontiguousarray(gwt.transpose(2, 1, 0, 3)).reshape(
        NCH, 128, NCH * 128).astype(BF16)
    cb_host = np.ascontiguousarray(conv_b.reshape(NCH, 128).T).astype(np.float32)
    gb_host = np.ascontiguousarray(gate_b.reshape(NCH, 128).T).astype(np.float32)
    return [
        _prep_core_input(x[BC * i:BC * (i + 1)], cw_host, gw_host, cb_host, gb_host)
        for i in range(NCORES)
    ]


def _unshard_core(o):
    # o: [D, RC] with columns ordered (group, t, win) -> [BC, T, D]
    return (o.reshape(D, NG, W, GWIN).transpose(1, 3, 2, 0)
             .reshape(NWIN, W, D).reshape(BC, T, D))


_NC_CACHE = None


def kernel(x, conv_w, conv_b, gate_w, gate_b):
    global _NC_CACHE
    x = np.asarray(x, np.float32)
    conv_w = np.asarray(conv_w, np.float32)
    conv_b = np.asarray(conv_b, np.float32)
    gate_w = np.asarray(gate_w, np.float32)
    gate_b = np.asarray(gate_b, np.float32)

    in_maps = _prep_in_maps(x, conv_w, conv_b, gate_w, gate_b)
    if _NC_CACHE is None:
        _NC_CACHE = _build()
    res = run_bass_kernel_spmd(_NC_CACHE, in_maps, core_ids=list(range(NCORES))).results

    out = np.empty((B, T, D), np.float32)
    for i in range(NCORES):
        out[BC * i:BC * (i + 1)] = _unshard_core(np.asarray(res[i]["outT"]))
    return out


# revision 8
# speedup vs baseline: 1.1095x; 1.0285x over previous
"""Trainium2 Bass kernel for windowed Conv1d(k=3) + sigmoid gating.

Reference computation (B=16, T=960, D=1024, W=10):
  windows of size 10 are conv'd independently with per-window zero pad 1:
    cnn[r, d] = sum_{k,c} conv_w[d, c, k] * xpad[r + k - 1, c] + conv_b[d]
    out = cnn * sigmoid(cnn @ gate_w.T + gate_b)

Strategy: pure data parallelism over the 8 NeuronCores (2 batches per
core, 192 windows = 1920 rows each). All layout work happens on the host:
  - x is window-padded and transposed to [D, 12, 192] (channel-major,
    tap-position-major) so the conv becomes 24 accumulating bf16 matmuls
    per output tile with the contraction dim on SBUF partitions; the 3
    taps are column offsets of k*192 elements (always 4B aligned).
  - conv_w / gate_w are pre-transposed into lhsT layout, bf16.
  - The kernel computes everything in transposed space [d, r] and writes a
    transposed f32 output that the host transposes back.
"""

import numpy as np
import ml_dtypes

import concourse.bacc as bacc
import concourse.bass as bass
import concourse.tile as tile
from concourse import mybir
from concourse.bass_utils import run_bass_kernel_spmd

BF16 = ml_dtypes.bfloat16

B, T, D, W = 16, 960, 1024, 10
NCORES = 8
BC = B // NCORES            # batches per core
NWIN = BC * T // W          # windows per core (192)
RC = NWIN * W               # output rows per core (1920)
PW = W + 2                  # padded window length (12)
NG = 4                      # column groups per core
GWIN = NWIN // NG           # windows per group (48)
GN = GWIN * W               # output columns per group (480)
NCH = D // 128              # 128-partition chunks of D (8)


def _build():
    nc = bacc.Bacc("TRN2", target_bir_lowering=False, debug=False)

    # xpt: [c, group, t*win] window-padded transposed input
    xpt = nc.dram_tensor("xpt", [D, NG, PW * GWIN], mybir.dt.bfloat16,
                         kind="ExternalInput")
    # cwr[dck]: [c_in_chunk, (k*NCH+ck)*128 + d_in_chunk] conv lhsT blocks
    cwr = nc.dram_tensor("cwr", [NCH, 128, 3 * NCH * 128], mybir.dt.bfloat16,
                         kind="ExternalInput")
    # gwr[eck]: [d_in_chunk, dck*128 + e_in_chunk] gate lhsT blocks
    gwr = nc.dram_tensor("gwr", [NCH, 128, NCH * 128], mybir.dt.bfloat16,
                         kind="ExternalInput")
    cb = nc.dram_tensor("cb", [128, NCH], mybir.dt.float32, kind="ExternalInput")
    gb = nc.dram_tensor("gb", [128, NCH], mybir.dt.float32, kind="ExternalInput")
    outT = nc.dram_tensor("outT", [D, RC], mybir.dt.float32, kind="ExternalOutput")

    with tile.TileContext(nc) as tc:
        with (
            tc.tile_pool(name="consts", bufs=1) as consts,
            tc.tile_pool(name="work", bufs=3) as work,
            tc.tile_pool(name="cnn", bufs=2) as cnnp,
            tc.tile_pool(name="cpsum", bufs=2, space="PSUM") as cpsum,
            tc.tile_pool(name="gpsum", bufs=2, space="PSUM") as gpsum,
        ):
            # DMA issue order = first-use order, split over the two HWDGE
            # queues (Sync: weights; Scalar: x) so the first conv tile's
            # dependencies land in ~10us instead of serializing on one queue.
            xpt_sb = [[None] * NG for _ in range(NCH)]

            def load_x(c, g):
                t = consts.tile([128, PW * GWIN], mybir.dt.bfloat16, tag=f"x{c}g{g}")
                nc.scalar.dma_start(t[:], xpt[c * 128:(c + 1) * 128, g])
                xpt_sb[c][g] = t

            cwr_sb = [None] * NCH

            def load_cw(dck):
                t = consts.tile([128, 3 * NCH * 128], mybir.dt.bfloat16, tag=f"cw{dck}")
                nc.sync.dma_start(t[:], cwr[dck])
                cwr_sb[dck] = t

            load_cw(0)
            for c in range(NCH):
                load_x(c, 0)

            cb_sb = consts.tile([128, NCH], mybir.dt.float32, tag="cb")
            nc.sync.dma_start(cb_sb[:], cb[:])
            gb_sb = consts.tile([128, NCH], mybir.dt.float32, tag="gb")
            nc.sync.dma_start(gb_sb[:], gb[:])

            for dck in range(1, NCH):
                load_cw(dck)

            gwr_sb = []
            for eck in range(NCH):
                t = consts.tile([128, NCH * 128], mybir.dt.bfloat16, tag=f"gw{eck}")
                nc.sync.dma_start(t[:], gwr[eck])
                gwr_sb.append(t)

            for g in range(1, NG):
                for c in range(NCH):
                    load_x(c, g)

            for g in range(NG):
                # conv: cnnT[d, (t, win)] = sum_{c,k} cw[k][c, d] * xpt[c, t+k, win]
                cnnT = []
                for dck in range(NCH):
                    ps = cpsum.tile([128, GN], mybir.dt.float32, tag="cps")
                    n_mm = 0
                    for ck in range(NCH):
                        xg = xpt_sb[ck][g][:].rearrange("p (t w) -> p t w", w=GWIN)
                        for k in range(3):
                            j = k * NCH + ck
                            nc.tensor.matmul(
                                ps[:],
                                cwr_sb[dck][:, j * 128:(j + 1) * 128],
                                xg[:, k:k + W, :],
                                start=(n_mm == 0),
                                stop=(n_mm == 3 * NCH - 1),
                            )
                            n_mm += 1
                    ct = cnnp.tile([128, GN], mybir.dt.bfloat16, tag=f"cnn{dck}")
                    nc.vector.tensor_scalar_add(ct[:], ps[:], cb_sb[:, dck:dck + 1])
                    cnnT.append(ct)

                # gate: gateT[e, r] = sigmoid(sum_d gw[d, e] * cnnT[d, r] + gb[e])
                for eck in range(NCH):
                    ps2 = gpsum.tile([128, GN], mybir.dt.float32, tag="gps")
                    for dck in range(NCH):
                        nc.tensor.matmul(
                            ps2[:],
                            gwr_sb[eck][:, dck * 128:(dck + 1) * 128],
                            cnnT[dck][:],
                            start=(dck == 0),
                            stop=(dck == NCH - 1),
                        )
                    gt = work.tile([128, GN], mybir.dt.bfloat16, tag="gate")
                    nc.scalar.activation(
                        gt[:], ps2[:], mybir.ActivationFunctionType.Sigmoid,
                        bias=gb_sb[:, eck:eck + 1],
                    )
                    ot = work.tile([128, GN], mybir.dt.float32, tag="out")
                    nc.vector.tensor_mul(ot[:], cnnT[eck][:], gt[:])
                    nc.sync.dma_start(
                        outT[eck * 128:(eck + 1) * 128, g * GN:(g + 1) * GN], ot[:]
                    )
    nc.compile()
    return nc


def _prep_core_input(x_shard, cw_host, gw_host, cb_host, gb_host):
    # x_shard: [BC, T, D] -> padded transposed [D, NG, PW, GWIN]
    xs = x_shard.reshape(NG, GWIN, W, D)
    xp = np.zeros((D, NG, PW, GWIN), np.float32)
    xp[:, :, 1:1 + W, :] = xs.transpose(3, 0, 2, 1)
    xpt_host = np.ascontiguousarray(xp).astype(BF16).reshape(D, NG, PW * GWIN)
    return {"xpt": xpt_host, "cwr": cw_host, "gwr": gw_host,
            "cb": cb_host, "gb": gb_host}


def _prep_in_maps(x, conv_w, conv_b, gate_w, gate_b):
    # conv lhsT blocks: cwr[dck][cc, (k*NCH+ck)*128 + dd] = conv_w[dck*128+dd, ck*128+cc, k]
    cwt = conv_w.transpose(2, 1, 0).reshape(3, NCH, 128, NCH, 128)  # [k, ck, cc, dck, dd]
    cw_host = np.ascontiguousarray(cwt.transpose(3, 2, 0, 1, 4)).reshape(
        NCH, 128, 3 * NCH * 128).astype(BF16)
    # gate lhsT blocks: gwr[eck][dd, dck*128 + ee] = gate_w[eck*128+ee, dck*128+dd]
    gwt = gate_w.T.reshape(NCH, 128, NCH, 128)  # [dck, dd, eck, ee]
    gw_host = np.ascontiguousarray(gwt.transpose(2, 1, 0, 3)).reshape(
        NCH, 128, NCH * 128).astype(BF16)
    cb_host = np.ascontiguousarray(conv_b.reshape(NCH, 128).T).astype(np.float32)
    gb_host = np.ascontiguousarray(gate_b.reshape(NCH, 128).T).astype(np.float32)
    return [
        _prep_core_input(x[BC * i:BC * (i + 1)], cw_host, gw_host, cb_host, gb_host)
        for i in range(NCORES)
    ]


def _unshard_core(o):
    # o: [D, RC] with columns ordered (group, t, win) -> [BC, T, D]
    return (o.reshape(D, NG, W, GWIN).transpose(1, 3, 2, 0)
             .reshape(NWIN, W, D).reshape(BC, T, D))


_NC_CACHE = None


def kernel(x, conv_w, conv_b, gate_w, gate_b):
    global _NC_CACHE
    x = np.asarray(x, np.float32)
    conv_w = np.asarray(conv_w, np.float32)
    conv_b = np.asarray(conv_b, np.float32)
    gate_w = np.asarray(gate_w, np.float32)
    gate_b = np.asarray(gate_b, np.float32)

    in_maps = _prep_in_maps(x, conv_w, conv_b, gate_w, gate_b)
    if _NC_CACHE is None:
        _NC_CACHE = _build()
    res = run_bass_kernel_spmd(_NC_CACHE, in_maps, core_ids=list(range(NCORES))).results

    out = np.empty((B, T, D), np.float32)
    for i in range(NCORES):
        out[BC * i:BC * (i + 1)] = _unshard_core(np.asarray(res[i]["outT"]))
    return out


# revision 10
# speedup vs baseline: 1.3133x; 1.1837x over previous
"""Trainium2 Bass kernel for windowed Conv1d(k=3) + sigmoid gating.

Reference computation (B=16, T=960, D=1024, W=10):
  windows of size 10 are conv'd independently with per-window zero pad 1:
    cnn[t, d] = sum_{k,c} conv_w[d, c, k] * xpad[t + k, c] + conv_b[d]
    out = cnn * sigmoid(cnn @ gate_w.T + gate_b)

Strategy: pure data parallelism over the 8 NeuronCores (2 batches per
core, 192 windows = 1920 rows each), with the conv done as Winograd
F(2,3): for each output pair (y0, y1) = (t=2p, 2p+1) with inputs
x0..x3 = xpad[2p..2p+3]:
    m1 = Wt0 x~0   m2 = Wt1 x~1   m3 = Wt2 x~2   m4 = Wt3 x~3
    y0 = m1 + m2 + m3          y1 = m2 - m3 - m4
    Wt = [W0, (W0+W1+W2)/2, (W0-W1+W2)/2, W2]
    x~ = [x0-x2, x1+x2, x2-x1, x1-x3]
The x~ transforms are precomputed on the host (so is the padding, the
transpose to channel-major layout, and the weight transform/transpose
into lhsT blocks, all in f32 then cast to bf16). The PE does 4 N=240
matmul streams per pair-group instead of 6 (2 outputs x 3 taps), a 1.5x
FLOP reduction; the A^T output combine runs on ScalarE/VectorE under
the matmul stream. Everything on-chip is computed in transposed space
[d, r] as bf16 with f32 PSUM accumulation; the host transposes the f32
output back.
"""

import numpy as np
import ml_dtypes

import concourse.bacc as bacc
import concourse.bass as bass
import concourse.tile as tile
from concourse import mybir
from concourse.bass_utils import run_bass_kernel_spmd

BF16 = ml_dtypes.bfloat16

B, T, D, W = 16, 960, 1024, 10
NCORES = 8
BC = B // NCORES            # batches per core
NWIN = BC * T // W          # windows per core (192)
RC = NWIN * W               # output rows per core (1920)
PW = W + 2                  # padded window length (12)
NP = W // 2                 # winograd output pairs per window (5)
NG = 4                      # column groups per core
GWIN = NWIN // NG           # windows per group (48)
GN = GWIN * W               # output columns per group (480)
GM = NP * GWIN              # winograd columns per group (240)
NCH = D // 128              # 128-partition chunks of D (8)
AF = mybir.ActivationFunctionType


def _build():
    nc = bacc.Bacc("TRN2", target_bir_lowering=False, debug=False)

    # xt: [c, group, (j, pair, win)] winograd-transformed input
    xt = nc.dram_tensor("xt", [D, NG, 4 * GM], mybir.dt.bfloat16,
                        kind="ExternalInput")
    # cwr[dck]: [cc, ((j*NCH+ck)*128 + dd)] winograd conv lhsT blocks
    cwr = nc.dram_tensor("cwr", [NCH, 128, 4 * NCH * 128], mybir.dt.bfloat16,
                         kind="ExternalInput")
    # gwr[eck]: [dd, (dck*128 + ee)] gate lhsT blocks
    gwr = nc.dram_tensor("gwr", [NCH, 128, NCH * 128], mybir.dt.bfloat16,
                         kind="ExternalInput")
    cb = nc.dram_tensor("cb", [128, NCH], mybir.dt.float32, kind="ExternalInput")
    gb = nc.dram_tensor("gb", [128, NCH], mybir.dt.float32, kind="ExternalInput")
    outT = nc.dram_tensor("outT", [D, RC], mybir.dt.float32, kind="ExternalOutput")

    with tile.TileContext(nc) as tc:
        with (
            tc.tile_pool(name="consts", bufs=1) as consts,
            tc.tile_pool(name="work", bufs=3) as work,
            tc.tile_pool(name="cnn", bufs=2) as cnnp,
            tc.tile_pool(name="cpsum", bufs=3, space="PSUM") as cpsum,
            tc.tile_pool(name="gpsum", bufs=2, space="PSUM") as gpsum,
        ):
            # DMA issue order = first-use order, split over the two HWDGE
            # queues (Sync: weights; Scalar: x~). Group g+1's x~ issues are
            # interleaved into group g's compute so ScalarE stays available
            # for the winograd epilogue copies.
            xt_sb = [[None] * NG for _ in range(NCH)]

            def load_x(c, g):
                t = consts.tile([128, 4 * GM], mybir.dt.bfloat16, tag=f"x{c}g{g}")
                nc.scalar.dma_start(t[:], xt[c * 128:(c + 1) * 128, g])
                xt_sb[c][g] = t

            cwr_sb = [None] * NCH

            def load_cw(dck):
                t = consts.tile([128, 4 * NCH * 128], mybir.dt.bfloat16,
                                tag=f"cw{dck}")
                nc.sync.dma_start(t[:], cwr[dck])
                cwr_sb[dck] = t

            load_cw(0)
            for c in range(NCH):
                load_x(c, 0)

            cb_sb = consts.tile([128, NCH], mybir.dt.float32, tag="cb")
            nc.sync.dma_start(cb_sb[:], cb[:])
            gb_sb = consts.tile([128, NCH], mybir.dt.float32, tag="gb")
            nc.sync.dma_start(gb_sb[:], gb[:])

            for dck in range(1, NCH):
                load_cw(dck)

            gwr_sb = []
            for eck in range(NCH):
                t = consts.tile([128, NCH * 128], mybir.dt.bfloat16, tag=f"gw{eck}")
                nc.sync.dma_start(t[:], gwr[eck])
                gwr_sb.append(t)

            for g in range(NG):
                # conv (winograd): m_j[d, (p, win)] accumulated over c-chunks
                cnnT = []
                for dck in range(NCH):
                    if g + 1 < NG and dck < NCH:
                        load_x(dck, g + 1)
                    ps = cpsum.tile([128, 4, 256], mybir.dt.float32, tag="cps")
                    # j outer: a start=True clears has_written for its whole
                    # PSUM bank, so the two j-groups sharing a bank must not
                    # interleave their accumulation.
                    for j in range(4):
                        for ck in range(NCH):
                            nc.tensor.matmul(
                                ps[:, j, :GM],
                                cwr_sb[dck][:, (j * NCH + ck) * 128:
                                            (j * NCH + ck + 1) * 128],
                                xt_sb[ck][g][:, j * GM:(j + 1) * GM],
                                start=(ck == 0),
                                stop=(ck == NCH - 1),
                            )
                    # A^T combine: y0 = m1+m2+m3+cb, y1 = m2-m3-m4+cb
                    cbs = cb_sb[:, dck:dck + 1]
                    m2s = work.tile([128, GM], mybir.dt.bfloat16, tag="m2s")
                    nc.scalar.activation(m2s[:], ps[:, 1, :GM], AF.Copy)
                    m3s = work.tile([128, GM], mybir.dt.bfloat16, tag="m3s")
                    nc.scalar.activation(m3s[:], ps[:, 2, :GM], AF.Copy)
                    t0 = work.tile([128, GM], mybir.dt.bfloat16, tag="t0")
                    nc.vector.tensor_scalar_add(t0[:], ps[:, 0, :GM], cbs)
                    u = work.tile([128, GM], mybir.dt.bfloat16, tag="u")
                    nc.vector.tensor_scalar(u[:], ps[:, 3, :GM], cbs, None,
                                            mybir.AluOpType.subtract)
                    ct = cnnp.tile([128, GN], mybir.dt.bfloat16, tag=f"cnn{dck}")
                    ctv = ct[:].rearrange("q (p two w) -> q two p w",
                                          two=2, w=GWIN)
                    a = work.tile([128, GM], mybir.dt.bfloat16, tag="a")
                    nc.vector.tensor_add(a[:], t0[:], m2s[:])
                    nc.vector.tensor_add(ctv[:, 0], a[:], m3s[:])
                    v = work.tile([128, GM], mybir.dt.bfloat16, tag="v")
                    nc.vector.tensor_sub(v[:], m2s[:], m3s[:])
                    nc.vector.tensor_sub(ctv[:, 1], v[:], u[:])
                    cnnT.append(ct)

                # gate: gateT[e, r] = sigmoid(sum_d gw[d, e] * cnnT[d, r] + gb[e])
                for eck in range(NCH):
                    ps2 = gpsum.tile([128, GN], mybir.dt.float32, tag="gps")
                    for dck in range(NCH):
                        nc.tensor.matmul(
                            ps2[:],
                            gwr_sb[eck][:, dck * 128:(dck + 1) * 128],
                            cnnT[dck][:],
                            start=(dck == 0),
                            stop=(dck == NCH - 1),
                        )
                    gt = work.tile([128, GN], mybir.dt.bfloat16, tag="gate")
                    nc.scalar.activation(gt[:], ps2[:], AF.Sigmoid,
                                         bias=gb_sb[:, eck:eck + 1])
                    ot = work.tile([128, GN], mybir.dt.float32, tag="out")
                    nc.vector.tensor_mul(ot[:], cnnT[eck][:], gt[:])
                    nc.sync.dma_start(
                        outT[eck * 128:(eck + 1) * 128, g * GN:(g + 1) * GN], ot[:]
                    )
    nc.compile()
    return nc


def _prep_core_input(x_shard, cw_host, gw_host, cb_host, gb_host):
    # x_shard: [BC, T, D] -> padded transposed [D, NG, PW, GWIN]
    xs = x_shard.reshape(NG, GWIN, W, D)
    xp = np.zeros((D, NG, PW, GWIN), np.float32)
    xp[:, :, 1:1 + W, :] = xs.transpose(3, 0, 2, 1)
    # winograd input transform, pairs p: x_i = xp[2p + i]
    x0 = xp[:, :, 0:2 * NP:2]
    x1 = xp[:, :, 1:1 + 2 * NP:2]
    x2 = xp[:, :, 2:2 + 2 * NP:2]
    x3 = xp[:, :, 3:3 + 2 * NP:2]
    xt = np.stack([x0 - x2, x1 + x2, x2 - x1, x1 - x3], axis=2)  # [D,NG,4,NP,GWIN]
    xt_host = np.ascontiguousarray(xt).astype(BF16).reshape(D, NG, 4 * GM)
    return {"xt": xt_host, "cwr": cw_host, "gwr": gw_host,
            "cb": cb_host, "gb": gb_host}


def _prep_in_maps(x, conv_w, conv_b, gate_w, gate_b):
    # winograd weight transform + lhsT blocks:
    # cwr[dck][cc, (j*NCH+ck)*128 + dd] = Wt_j[dck*128+dd, ck*128+cc]
    W0, W1, W2 = conv_w[:, :, 0], conv_w[:, :, 1], conv_w[:, :, 2]
    wt = np.stack([W0, (W0 + W1 + W2) * 0.5, (W0 - W1 + W2) * 0.5, W2])  # [4,d,c]
    wt = wt.reshape(4, NCH, 128, NCH, 128)  # [j, dck, dd, ck, cc]
    cw_host = np.ascontiguousarray(wt.transpose(1, 4, 0, 3, 2)).reshape(
        NCH, 128, 4 * NCH * 128).astype(BF16)
    # gate lhsT blocks: gwr[eck][dd, dck*128 + ee] = gate_w[eck*128+ee, dck*128+dd]
    gwt = gate_w.T.reshape(NCH, 128, NCH, 128)  # [dck, dd, eck, ee]
    gw_host = np.ascontiguousarray(gwt.transpose(2, 1, 0, 3)).reshape(
        NCH, 128, NCH * 128).astype(BF16)
    cb_host = np.ascontiguousarray(conv_b.reshape(NCH, 128).T).astype(np.float32)
    gb_host = np.ascontiguousarray(gate_b.reshape(NCH, 128).T).astype(np.float32)
    return [
        _prep_core_input(x[BC * i:BC * (i + 1)], cw_host, gw_host, cb_host, gb_host)
        for i in range(NCORES)
    ]


def _unshard_core(o):
    # o: [D, RC] with columns ordered (group, t, win) -> [BC, T, D]
    return (o.reshape(D, NG, W, GWIN).transpose(1, 3, 2, 0)
             .reshape(NWIN, W, D).reshape(BC, T, D))


_NC_CACHE = None


def kernel(x, conv_w, conv_b, gate_w, gate_b):
    global _NC_CACHE
    x = np.asarray(x, np.float32)
    conv_w = np.asarray(conv_w, np.float32)
    conv_b = np.asarray(conv_b, np.float32)
    gate_w = np.asarray(gate_w, np.float32)
    gate_b = np.asarray(gate_b, np.float32)

    in_maps = _prep_in_maps(x, conv_w, conv_b, gate_w, gate_b)
    if _NC_CACHE is None:
        _NC_CACHE = _build()
    res = run_bass_kernel_spmd(_NC_CACHE, in_maps, core_ids=list(range(NCORES))).results

    out = np.empty((B, T, D), np.float32)
    for i in range(NCORES):
        out[BC * i:BC * (i + 1)] = _unshard_core(np.asarray(res[i]["outT"]))
    return out
